# revision 19
# baseline (speedup 1.0000x reference)
"""Trainium2 Bass kernel for the BMN-style network (nn_BMN_481036337693).

Strategy:
  - Fuse the huge `bm = cm @ sample_mask` matmul (B,256,320000) and the
    following 3D conv (stride NS) into:  V[(n,t), o] = sum_c p3d_w[o,c,n] cm[c,t]
    then  y[o, (s,e)] = sum_(n,t) V[(n,t), o] * W2[(n,t), (s,e)]
    where W2 is sample_mask reshaped to (NS*T, T*T).  This cuts ~200 GFLOP
    to ~67 GFLOP.
  - Shard 8 cores = batch (2) x row-bands of the (s,e) image (4 bands of
    25 rows + 2-row conv halos -> 29 rows each).  All cores run one SPMD
    program; per-core differences (image-boundary validity) are data-driven
    via two row-mask scalars (m_lo, m_hi) multiplied onto boundary rows.
  - fp32 matmuls stream at half rate on TRN2, so everything on the heavy
    path (W2, V, p3d, Y, y1, y2, q-weights) is bf16; PSUM accumulation and
    the small 1D stack / final layers stay fp32.
"""

import numpy as np
import ml_dtypes

T, NS, NP, FEAT = 100, 32, 3, 400
H1, H2, H3 = 256, 128, 512
B = 2
ROWS = 29            # 25 output rows + 2 halo rows each side
PIX = ROWS * T       # 2900
KT = (NS * T) // 128  # 25 k-tiles of the fused contraction
NBLK = 6             # big-matmul pixel blocks of 512 (last 340)

# fp32 const pack layout (columns)
_PF = {}
_off = 0
for _nm, _w in (("b2w", 384), ("s1w", 384), ("e1w", 384), ("p1w", 1536),
                ("b1b", 2), ("b2b", 2), ("s1b", 2), ("e1b", 2), ("p1b", 2),
                ("s2w", 2), ("e2w", 2), ("s2b", 1), ("e2b", 1),
                ("p3db", 4), ("q1b", 1), ("q2b", 1), ("q3b", 1),
                ("q4b", 1), ("rmrows", 2)):
    _PF[_nm] = (_off, _w)
    _off += _w
NF = _off
# bf16 const pack layout
_PH = {"q1w": (0, 512), "q2w": (512, 1152), "q3w": (1664, 1152), "q4w": (2816, 2)}
NH = 2818

_COMPILED = None


def _build():
    import concourse.bacc as bacc
    import concourse.mybir as mybir
    import concourse.tile as tile

    AF = mybir.ActivationFunctionType
    F32 = mybir.dt.float32
    BF16 = mybir.dt.bfloat16

    nc = bacc.Bacc("TRN2", target_bir_lowering=False, debug=False, num_devices=8)

    def din(name, shape, dt=F32):
        return nc.dram_tensor(name, shape, dt, kind="ExternalInput").ap()

    pack0_d = din("pack0", (128, 6))
    xg_d = din("xg", (100, 4, 102))
    b1w_d = din("b1w", (100, 4, 3, 64))
    packf_d = din("packf", (128, NF))
    packh_d = din("packh", (128, NH), BF16)
    p3d_d = din("p3d", (128, NS, 2, 512), BF16)
    w2_d = din("w2", (NBLK, 128, KT, 512), BF16)

    conf_d = nc.dram_tensor("conf_o", (2, 25 * T), F32, kind="ExternalOutput").ap()
    start_d = nc.dram_tensor("start_o", (1, T), F32, kind="ExternalOutput").ap()
    end_d = nc.dram_tensor("end_o", (1, T), F32, kind="ExternalOutput").ap()

    with tile.TileContext(nc) as tc:
        with tc.tile_pool(name="const", bufs=1) as cp:
            p0t = cp.tile([128, 6], F32, name="pack0_s")
            nc.sync.dma_start(out=p0t[:], in_=pack0_d[:])
            xg = cp.tile([100, 4, 102], F32, name="xg_s")
            nc.sync.dma_start(out=xg[:], in_=xg_d[:])
            b1w = cp.tile([100, 4, 3, 64], F32, name="b1w_s")
            nc.sync.dma_start(out=b1w[:], in_=b1w_d[:])
            pf = cp.tile([128, NF], F32, name="packf_s")
            nc.sync.dma_start(out=pf[:], in_=packf_d[:])
            ph = cp.tile([128, NH], BF16, name="packh_s")
            nc.scalar.dma_start(out=ph[:], in_=packh_d[:])

            def _view(ap, dims):
                if not dims:
                    return ap
                names = " ".join(f"d{i}" for i in range(len(dims) + 1))
                kw = {f"d{i}": d for i, d in enumerate(dims)}
                return ap.rearrange(f"p ({names}) -> p {names}", **kw)

            def pfv(nm, *dims):
                o, w = _PF[nm]
                return _view(pf[:, o:o + w], dims)

            def phv(nm, *dims):
                o, w = _PH[nm]
                return _view(ph[:, o:o + w], dims)

            b2w = pfv("b2w", 2, 3)      # [128, 2, 3, 64]
            s1w = pfv("s1w", 2, 3)
            e1w = pfv("e1w", 2, 3)
            p1w = pfv("p1w", 2, 3)      # [128, 2, 3, 256]
            b1b = p0t[:, 0:2]
            b2b = p0t[:, 2:4]
            s1b = pfv("s1b")
            e1b = pfv("e1b")
            p1b = p0t[:, 4:6]
            s2w = pfv("s2w")
            e2w = pfv("e2w")
            s2b = pfv("s2b")
            e2b = pfv("e2b")
            p3db = pfv("p3db")          # [128, 4]
            q1b = pfv("q1b")
            q2b = pfv("q2b")
            q3b = pfv("q3b")
            q4b = pfv("q4b")
            rmro = pfv("rmrows")        # [128, 2]: (m_lo, m_hi)

            q4w = phv("q4w")            # [128, 2] bf16
            q1w = phv("q1w", 4)         # [128, 4, 128] bf16
            q2w = phv("q2w", 9)         # [128, 9, 128] bf16
            q3w = phv("q3w", 9)

            # persistent activations
            base1 = [cp.tile([128, 102], F32, name=f"base1_{j}") for j in range(2)]
            basef = [cp.tile([128, 102], F32, name=f"basef_{j}") for j in range(2)]
            sr = [cp.tile([128, 100], F32, name=f"sr_{j}") for j in range(2)]
            er = [cp.tile([128, 100], F32, name=f"er_{j}") for j in range(2)]
            cm = [cp.tile([128, 100], BF16, name=f"cm_{j}") for j in range(2)]
            vsb = cp.tile([128, KT, 512], BF16, name="vsb")
            ysb = [cp.tile([128, PIX], BF16, name=f"ysb_{m}") for m in range(4)]
            y1 = cp.tile([128, ROWS, 102], BF16, name="y1")
            y2 = cp.tile([128, ROWS, 102], BF16, name="y2")
            y3 = cp.tile([128, 25 * T], BF16, name="y3")
            conf_s = cp.tile([2, 25 * T], F32, name="conf_s")
            start_s = cp.tile([1, T], F32, name="start_s")
            end_s = cp.tile([1, T], F32, name="end_s")

            for t_ in (base1[0], base1[1], basef[0], basef[1]):
                nc.vector.memset(t_[:, 0:1], 0.0)
                nc.vector.memset(t_[:, 101:102], 0.0)
            for t_ in (y1, y2):
                nc.vector.memset(t_[:, :, 0:1], 0.0)
                nc.vector.memset(t_[:, :, 101:102], 0.0)

            # ---------------- 1D conv stack: critical path to cm ----------------
            with tc.tile_pool(name="ps1d", bufs=4, space="PSUM") as pp:
                for j in range(2):
                    ps = pp.tile([128, 100], F32, name=f"ps_b1_{j}", tag="ps1d")
                    for g2 in range(2):
                        g = 2 * j + g2
                        for dt in range(3):
                            nc.tensor.matmul(
                                ps[64 * g2:64 * g2 + 64, :],
                                b1w[:, g, dt, :],
                                xg[:, g, dt:dt + 100],
                                start=(dt == 0), stop=(dt == 2))
                    nc.scalar.activation(base1[j][:, 1:101], ps[:], AF.Relu,
                                         bias=b1b[:, j:j + 1])
                for j in range(2):
                    ps = pp.tile([128, 100], F32, name=f"ps_b2_{j}", tag="ps1d")
                    for g2 in range(2):
                        sl = slice(64 * g2, 64 * g2 + 64)
                        for dt in range(3):
                            nc.tensor.matmul(
                                ps[sl, :],
                                b2w[sl, j, dt, :],
                                base1[j][sl, dt:dt + 100],
                                start=(dt == 0), stop=(dt == 2))
                    nc.scalar.activation(basef[j][:, 1:101], ps[:], AF.Relu,
                                         bias=b2b[:, j:j + 1])
                # p1 -> cm (bf16)
                for m in range(2):
                    ps = pp.tile([128, 100], F32, name=f"ps_p1_{m}", tag="ps1d")
                    for j in range(2):
                        for dt in range(3):
                            nc.tensor.matmul(
                                ps[:],
                                p1w[:, j, dt, 128 * m:128 * m + 128],
                                basef[j][:, dt:dt + 100],
                                start=(j == 0 and dt == 0), stop=(j == 1 and dt == 2))
                    nc.scalar.activation(cm[m][:], ps[:], AF.Relu,
                                         bias=p1b[:, m:m + 1])

                # ---- start/end branches: interleaved to overlap ACT chains ----
                ps_se = {}
                for bi, (wte, bte, dest) in enumerate(
                        ((s1w, s1b, sr), (e1w, e1b, er))):
                    for j in range(2):
                        ps = pp.tile([128, 100], F32, name=f"ps_se_{bi}_{j}",
                                     tag="ps1d")
                        ps_se[(bi, j)] = ps
                        for g2 in range(2):
                            sl = slice(64 * g2, 64 * g2 + 64)
                            for dt in range(3):
                                nc.tensor.matmul(
                                    ps[sl, :],
                                    wte[sl, j, dt, :],
                                    basef[j][sl, dt:dt + 100],
                                    start=(dt == 0), stop=(dt == 2))
                for bi, (bte, dest) in enumerate(((s1b, sr), (e1b, er))):
                    for j in range(2):
                        nc.scalar.activation(dest[j][:], ps_se[(bi, j)][:], AF.Relu,
                                             bias=bte[:, j:j + 1])
                ps1s = []
                for bi, (dest, w2e) in enumerate(((sr, s2w), (er, e2w))):
                    ps1 = pp.tile([1, 100], F32, name=f"ps_1_{bi}", tag="ps1", bufs=2)
                    ps1s.append(ps1)
                    for j in range(2):
                        nc.tensor.matmul(ps1[0:1, :], w2e[:, j:j + 1], dest[j][:],
                                         start=(j == 0), stop=(j == 1))
                for bi, (b2e, outdram, stile) in enumerate(
                        ((s2b, start_d, start_s), (e2b, end_d, end_s))):
                    nc.scalar.activation(stile[:], ps1s[bi][0:1, :], AF.Sigmoid,
                                         bias=b2e[0:1, 0:1])
                    nc.scalar.dma_start(out=outdram[:], in_=stile[:])

            # ------- big matmul: y = V^T @ W2, relu(+bias); V built in block 0 -------
            KC = 5    # k-tiles per W2 DMA
            NCHUNK = 4
            LA = 2    # V lookahead (k-tiles) in block 0
            with tc.tile_pool(name="w2st", bufs=4) as w2p, \
                 tc.tile_pool(name="psy", bufs=6, space="PSUM") as pyp, \
                 tc.tile_pool(name="p3dst", bufs=8) as p3p, \
                 tc.tile_pool(name="vstg", bufs=4) as vsp, \
                 tc.tile_pool(name="psv", bufs=2, space="PSUM") as pvp:
                p3ts = []
                for n0 in range(0, NS, NCHUNK):
                    p3t_ = p3p.tile([128, NCHUNK, 2, 512], BF16,
                                    name=f"p3t{n0}", tag="p3t")
                    nc.sync.dma_start(out=p3t_[:], in_=p3d_d[:, n0:n0 + NCHUNK, :, :])
                    p3ts.append(p3t_)

                def emit_v(n):
                    p3t = p3ts[n // NCHUNK]
                    psv = pvp.tile([100, 512], F32, name="psv", tag="psv")
                    for j in range(2):
                        nc.tensor.matmul(psv[0:100, :], cm[j][:, 0:100],
                                         p3t[:, n % NCHUNK, j, :],
                                         start=(j == 0), stop=(j == 1))
                    vst = vsp.tile([100, 512], BF16, name="vst", tag="vst")
                    nc.vector.tensor_copy(vst[:], psv[0:100, :])
                    nt0 = 100 * n
                    k0, p0 = divmod(nt0, 128)
                    seg = min(128 - p0, 100)
                    eng = (nc.scalar, nc.sync)[n % 2]
                    eng.dma_start(out=vsb[p0:p0 + seg, k0, :], in_=vst[0:seg, :])
                    if seg < 100:
                        eng.dma_start(out=vsb[0:100 - seg, k0 + 1, :],
                                      in_=vst[seg:100, :])

                navail = 0
                for blk in range(NBLK):
                    c0 = 512 * blk
                    N = min(512, PIX - c0)
                    psy = [pyp.tile([128, 512], F32, name=f"psy{m}", tag="psy")
                           for m in range(4)]
                    w2t = None
                    for k in range(KT):
                        while navail < NS and navail <= ((k + LA) * 128 + 127) // 100:
                            emit_v(navail)
                            navail += 1
                        if k % KC == 0:
                            w2t = w2p.tile([128, KC, 512], BF16, name="w2t", tag="w2t")
                            w2eng = nc.scalar if blk == 0 else nc.sync
                            w2eng.dma_start(
                                out=w2t[:],
                                in_=w2_d[blk, :, k:k + KC, :])
                        for m in range(4):
                            nc.tensor.matmul(
                                psy[m][:, 0:N],
                                vsb[:, k, 128 * m:128 * m + 128],
                                w2t[:, k % KC, 0:N],
                                start=(k == 0), stop=(k == KT - 1))
                    for m in range(4):
                        nc.scalar.activation(ysb[m][:, c0:c0 + N], psy[m][:, 0:N],
                                             AF.Relu, bias=p3db[:, m:m + 1])
                # zero out-of-image boundary rows (rows 0,1 and 27,28)
                for m in range(4):
                    nc.vector.tensor_scalar_mul(ysb[m][:, 0:200], ysb[m][:, 0:200],
                                                rmro[:, 0:1])
                    nc.vector.tensor_scalar_mul(ysb[m][:, 2700:2900],
                                                ysb[m][:, 2700:2900], rmro[:, 1:2])

            # ---------------- q1..q4 ----------------
            with tc.tile_pool(name="psq", bufs=6, space="PSUM") as pqp:
                # q1: 1x1, 512 -> 128, all 29 rows
                for c in range(6):
                    r0 = 5 * c
                    nr = min(5, ROWS - r0)
                    N = nr * 100
                    ps = pqp.tile([128, 500], F32, name="psq1", tag="psq")
                    for j in range(4):
                        nc.tensor.matmul(ps[:, 0:N], q1w[:, j, :],
                                         ysb[j][:, 100 * r0:100 * r0 + N],
                                         start=(j == 0), stop=(j == 3))
                    nc.scalar.activation(
                        y1[:, r0:r0 + nr, 1:101],
                        ps[:, 0:N].rearrange("p (r e) -> p r e", r=nr),
                        AF.Relu, bias=q1b[:, 0:1])
                nc.vector.tensor_scalar_mul(y1[:, 0:2, 1:101], y1[:, 0:2, 1:101],
                                            rmro[:, 0:1])
                nc.vector.tensor_scalar_mul(y1[:, 27:29, 1:101], y1[:, 27:29, 1:101],
                                            rmro[:, 1:2])
                # q2: 3x3 on y1 rows [0,29) -> y2 rows [1,28)
                chunks2 = [(1 + 5 * c, min(5, 28 - (1 + 5 * c))) for c in range(6)]
                ps2 = [pqp.tile([128, 500], F32, name=f"psq2_{c}", tag="psq")
                       for c in range(6)]
                for sh in range(9):
                    dr, dc = divmod(sh, 3)
                    for c, (r0, nr) in enumerate(chunks2):
                        N = nr * 100
                        nc.tensor.matmul(
                            ps2[c][:, 0:N],
                            q2w[:, sh, :],
                            y1[:, r0 + dr - 1:r0 + dr - 1 + nr, dc:dc + 100],
                            start=(sh == 0), stop=(sh == 8))
                for c, (r0, nr) in enumerate(chunks2):
                    N = nr * 100
                    nc.scalar.activation(
                        y2[:, r0:r0 + nr, 1:101],
                        ps2[c][:, 0:N].rearrange("p (r e) -> p r e", r=nr),
                        AF.Relu, bias=q2b[:, 0:1])
                nc.vector.tensor_scalar_mul(y2[:, 1:2, 1:101], y2[:, 1:2, 1:101],
                                            rmro[:, 0:1])
                nc.vector.tensor_scalar_mul(y2[:, 27:28, 1:101], y2[:, 27:28, 1:101],
                                            rmro[:, 1:2])
                # q3: 3x3 on y2 rows [1,28) -> y3 rows [2,27) (all valid)
                ps3 = [pqp.tile([128, 500], F32, name=f"psq3_{c}", tag="psq")
                       for c in range(5)]
                for sh in range(9):
                    dr, dc = divmod(sh, 3)
                    for c in range(5):
                        r0 = 2 + 5 * c
                        nc.tensor.matmul(
                            ps3[c][:, 0:500],
                            q3w[:, sh, :],
                            y2[:, r0 + dr - 1:r0 + dr - 1 + 5, dc:dc + 100],
                            start=(sh == 0), stop=(sh == 8))
                for c in range(5):
                    nc.scalar.activation(y3[:, 500 * c:500 * c + 500],
                                         ps3[c][:, 0:500], AF.Relu,
                                         bias=q3b[:, 0:1])
                # q4: 1x1 -> 2 ch, sigmoid
                for c in range(5):
                    ps4 = pqp.tile([2, 500], F32, name="psq4", tag="psq4", bufs=2)
                    nc.tensor.matmul(ps4[0:2, :], q4w[:, 0:2],
                                     y3[:, 500 * c:500 * c + 500],
                                     start=True, stop=True)
                    nc.scalar.activation(conf_s[0:2, 500 * c:500 * c + 500],
                                         ps4[0:2, :], AF.Sigmoid,
                                         bias=q4b[0:2, 0:1])
                nc.scalar.dma_start(out=conf_d[:], in_=conf_s[:])

    nc.compile()
    return nc


def _marshal(inputs):
    f32 = np.float32
    bf16 = ml_dtypes.bfloat16
    x = np.asarray(inputs["x"], f32)
    mask = np.asarray(inputs["sample_mask"], f32)

    xgs = []
    for b in range(B):
        xb = np.zeros((100, 4, 102), f32)
        xb[:, :, 1:101] = x[b].reshape(4, 100, 100).transpose(1, 0, 2)
        xgs.append(xb)

    b1w = np.zeros((100, 4, 3, 64), f32)
    w = np.asarray(inputs["b1_w"], f32)  # (256, 100, 3)
    for gi in range(4):
        b1w[:, gi, :, :] = w[64 * gi:64 * gi + 64].transpose(1, 2, 0)

    def group64(w):  # (256, 64, 3) -> (128, 2*3*64)
        out = np.zeros((128, 2, 3, 64), f32)
        for gi in range(4):
            out[64 * (gi % 2):64 * (gi % 2) + 64, gi // 2, :, :] = \
                w[64 * gi:64 * gi + 64].transpose(1, 2, 0)
        return out.reshape(128, -1)

    def bias2(b):
        return np.ascontiguousarray(np.asarray(b, f32).reshape(2, 128).T)

    pack0 = np.concatenate([bias2(inputs["b1_b"]), bias2(inputs["b2_b"]),
                            bias2(inputs["p1_b"])], axis=1)  # (128, 6)

    packf = np.zeros((128, NF), f32)

    def put(nm, arr):
        o, wdt = _PF[nm]
        packf[:arr.shape[0], o:o + wdt] = arr.reshape(arr.shape[0], wdt)

    put("b2w", group64(np.asarray(inputs["b2_w"], f32)))
    put("s1w", group64(np.asarray(inputs["s1_w"], f32)))
    put("e1w", group64(np.asarray(inputs["e1_w"], f32)))
    p1 = np.asarray(inputs["p1_w"], f32)  # (256, 256, 3)
    put("p1w", np.ascontiguousarray(
        p1.reshape(256, 2, 128, 3).transpose(2, 1, 3, 0)).reshape(128, -1))
    put("b1b", bias2(inputs["b1_b"]))
    put("b2b", bias2(inputs["b2_b"]))
    put("s1b", bias2(inputs["s1_b"]))
    put("e1b", bias2(inputs["e1_b"]))
    put("p1b", bias2(inputs["p1_b"]))
    put("s2w", np.ascontiguousarray(
        np.asarray(inputs["s2_w"], f32)[0, :, 0].reshape(2, 128).T))
    put("e2w", np.ascontiguousarray(
        np.asarray(inputs["e2_w"], f32)[0, :, 0].reshape(2, 128).T))
    packf[0, _PF["s2b"][0]] = np.asarray(inputs["s2_b"], f32).item()
    packf[0, _PF["e2b"][0]] = np.asarray(inputs["e2_b"], f32).item()
    put("p3db", np.ascontiguousarray(
        np.asarray(inputs["p3d_b"], f32).reshape(4, 128).T))
    packf[:, _PF["q1b"][0]] = np.asarray(inputs["q1_b"], f32)
    packf[:, _PF["q2b"][0]] = np.asarray(inputs["q2_b"], f32)
    packf[:, _PF["q3b"][0]] = np.asarray(inputs["q3_b"], f32)
    packf[0:2, _PF["q4b"][0]] = np.asarray(inputs["q4_b"], f32)

    packh = np.zeros((128, NH), bf16)
    q1 = np.asarray(inputs["q1_w"], f32)[:, :, 0, 0]  # (128, 512)
    packh[:, 0:512] = np.ascontiguousarray(
        q1.T.reshape(4, 128, 128).transpose(1, 0, 2)).reshape(128, 512).astype(bf16)
    for nm in ("q2", "q3"):
        qw = np.asarray(inputs[f"{nm}_w"], f32)
        o, wdt = _PH[f"{nm}w"]
        packh[:, o:o + wdt] = np.ascontiguousarray(
            qw.transpose(2, 3, 1, 0).reshape(9, 128, 128).transpose(1, 0, 2)
        ).reshape(128, wdt).astype(bf16)

    packh[:, 2816:2818] = np.ascontiguousarray(
        np.asarray(inputs["q4_w"], f32)[:, :, 0, 0].T).astype(bf16)

    p3 = np.asarray(inputs["p3d_w"], f32)[:, :, :, 0, 0]  # (512, 256, 32)
    p3dT = np.ascontiguousarray(
        p3.transpose(2, 1, 0).reshape(NS, 2, 128, 512).transpose(2, 0, 1, 3)
    ).astype(bf16)  # [c, n, j, o]

    w2_full = np.ascontiguousarray(
        mask.reshape(T, NS, T, T).transpose(1, 0, 2, 3)).reshape(NS * T, T, T)
    w2_bands, packfs = [], []
    for k in range(4):
        s_lo = 25 * k - 2
        w2c = np.zeros((NS * T, ROWS, T), f32)
        lo, hi = max(s_lo, 0), min(s_lo + ROWS, T)
        w2c[:, lo - s_lo:hi - s_lo, :] = w2_full[:, lo:hi, :]
        w2kt = w2c.reshape(KT, 128, PIX)
        w2b = np.zeros((NBLK, 128, KT, 512), np.float32)
        for blk in range(NBLK):
            c0 = 512 * blk
            n_ = min(512, PIX - c0)
            w2b[blk, :, :, 0:n_] = w2kt[:, :, c0:c0 + n_].transpose(1, 0, 2)
        w2_bands.append(w2b.astype(bf16))
        pfk = packf.copy()
        pfk[:, _PF["rmrows"][0]] = 1.0 if k > 0 else 0.0
        pfk[:, _PF["rmrows"][0] + 1] = 1.0 if k < 3 else 0.0
        packfs.append(pfk)

    in_maps = []
    for c in range(8):
        b, k = divmod(c, 4)
        in_maps.append({
            "pack0": pack0, "xg": xgs[b], "b1w": b1w, "packf": packfs[k],
            "packh": packh, "p3d": p3dT, "w2": w2_bands[k],
        })
    return in_maps


def kernel(**inputs):
    global _COMPILED
    from concourse.bass_utils import run_bass_kernel_spmd

    if _COMPILED is None:
        _COMPILED = _build()
    nc = _COMPILED

    in_maps = _marshal(inputs)
    res = run_bass_kernel_spmd(nc, in_maps, core_ids=list(range(8)),
                               trace=False)

    conf = np.zeros((B, 2, T, T), np.float32)
    start = np.zeros((B, T), np.float32)
    end = np.zeros((B, T), np.float32)
    for c in range(8):
        b, k = divmod(c, 4)
        r = res.results[c]
        conf[b, :, 25 * k:25 * k + 25, :] = r["conf_o"].reshape(2, 25, T)
        if k == 0:
            start[b] = r["start_o"][0]
            end[b] = r["end_o"][0]
    return conf, start, end


# revision 20
# speedup vs baseline: 1.0161x; 1.0161x over previous
"""Trainium2 Bass kernel for the BMN-style network (nn_BMN_481036337693).

Strategy:
  - Fuse the huge `bm = cm @ sample_mask` matmul (B,256,320000) and the
    following 3D conv (stride NS) into:  V[(n,t), o] = sum_c p3d_w[o,c,n] cm[c,t]
    then  y[o, (s,e)] = sum_(n,t) V[(n,t), o] * W2[(n,t), (s,e)]
    where W2 is sample_mask reshaped to (NS*T, T*T).  This cuts ~200 GFLOP
    to ~67 GFLOP.
  - Shard 8 cores = batch (2) x row-bands of the (s,e) image (4 bands of
    25 rows + 2-row conv halos -> 29 rows each).  All cores run one SPMD
    program; per-core differences (image-boundary validity) are data-driven
    via two row-mask scalars (m_lo, m_hi) multiplied onto boundary rows.
  - fp32 matmuls stream at half rate on TRN2, so everything on the heavy
    path (W2, V, p3d, Y, y1, y2, q-weights) is bf16; PSUM accumulation and
    the small 1D stack / final layers stay fp32.
"""

import numpy as np
import ml_dtypes

T, NS, NP, FEAT = 100, 32, 3, 400
H1, H2, H3 = 256, 128, 512
B = 2
ROWS = 29            # 25 output rows + 2 halo rows each side
PIX = ROWS * T       # 2900
KT = (NS * T) // 128  # 25 k-tiles of the fused contraction
NBLK = 6             # big-matmul pixel blocks of 512 (last 340)

# fp32 const pack layout (columns)
_PF = {}
_off = 0
for _nm, _w in (("b2w", 384), ("s1w", 384), ("e1w", 384), ("p1w", 1536),
                ("b1b", 2), ("b2b", 2), ("s1b", 2), ("e1b", 2), ("p1b", 2),
                ("s2w", 2), ("e2w", 2), ("s2b", 1), ("e2b", 1),
                ("p3db", 4), ("q1b", 1), ("q2b", 1), ("q3b", 1),
                ("q4b", 1), ("rmrows", 2)):
    _PF[_nm] = (_off, _w)
    _off += _w
NF = _off
# bf16 const pack layout
_PH = {"q1w": (0, 512), "q2w": (512, 1152), "q3w": (1664, 1152), "q4w": (2816, 2)}
NH = 2818

_COMPILED = None


def _build():
    import concourse.bacc as bacc
    import concourse.mybir as mybir
    import concourse.tile as tile

    AF = mybir.ActivationFunctionType
    F32 = mybir.dt.float32
    BF16 = mybir.dt.bfloat16

    nc = bacc.Bacc("TRN2", target_bir_lowering=False, debug=False, num_devices=8)

    def din(name, shape, dt=F32):
        return nc.dram_tensor(name, shape, dt, kind="ExternalInput").ap()

    pack0_d = din("pack0", (128, 6))
    xg_d = din("xg", (100, 4, 102))
    b1w_d = din("b1w", (100, 4, 3, 64))
    packf_d = din("packf", (128, NF))
    packh_d = din("packh", (128, NH), BF16)
    p3d_d = din("p3d", (128, NS, 2, 512), BF16)
    w2_d = din("w2", (NBLK, 128, KT, 512), BF16)

    conf_d = nc.dram_tensor("conf_o", (2, 25 * T), F32, kind="ExternalOutput").ap()
    start_d = nc.dram_tensor("start_o", (1, T), F32, kind="ExternalOutput").ap()
    end_d = nc.dram_tensor("end_o", (1, T), F32, kind="ExternalOutput").ap()

    with tile.TileContext(nc) as tc:
        with tc.tile_pool(name="const", bufs=1) as cp:
            p0t = cp.tile([128, 6], F32, name="pack0_s")
            nc.sync.dma_start(out=p0t[:], in_=pack0_d[:])
            xg = cp.tile([100, 4, 102], F32, name="xg_s")
            nc.sync.dma_start(out=xg[:], in_=xg_d[:])
            b1w = cp.tile([100, 4, 3, 64], F32, name="b1w_s")
            nc.sync.dma_start(out=b1w[:], in_=b1w_d[:])
            pf = cp.tile([128, NF], F32, name="packf_s")
            nc.scalar.dma_start(out=pf[:], in_=packf_d[:])
            ph = cp.tile([128, NH], BF16, name="packh_s")
            nc.scalar.dma_start(out=ph[:], in_=packh_d[:])

            def _view(ap, dims):
                if not dims:
                    return ap
                names = " ".join(f"d{i}" for i in range(len(dims) + 1))
                kw = {f"d{i}": d for i, d in enumerate(dims)}
                return ap.rearrange(f"p ({names}) -> p {names}", **kw)

            def pfv(nm, *dims):
                o, w = _PF[nm]
                return _view(pf[:, o:o + w], dims)

            def phv(nm, *dims):
                o, w = _PH[nm]
                return _view(ph[:, o:o + w], dims)

            b2w = pfv("b2w", 2, 3)      # [128, 2, 3, 64]
            s1w = pfv("s1w", 2, 3)
            e1w = pfv("e1w", 2, 3)
            p1w = pfv("p1w", 2, 3)      # [128, 2, 3, 256]
            b1b = p0t[:, 0:2]
            b2b = p0t[:, 2:4]
            s1b = pfv("s1b")
            e1b = pfv("e1b")
            p1b = p0t[:, 4:6]
            s2w = pfv("s2w")
            e2w = pfv("e2w")
            s2b = pfv("s2b")
            e2b = pfv("e2b")
            p3db = pfv("p3db")          # [128, 4]
            q1b = pfv("q1b")
            q2b = pfv("q2b")
            q3b = pfv("q3b")
            q4b = pfv("q4b")
            rmro = pfv("rmrows")        # [128, 2]: (m_lo, m_hi)

            q4w = phv("q4w")            # [128, 2] bf16
            q1w = phv("q1w", 4)         # [128, 4, 128] bf16
            q2w = phv("q2w", 9)         # [128, 9, 128] bf16
            q3w = phv("q3w", 9)

            # persistent activations
            base1 = [cp.tile([128, 102], F32, name=f"base1_{j}") for j in range(2)]
            basef = [cp.tile([128, 102], F32, name=f"basef_{j}") for j in range(2)]
            sr = [cp.tile([128, 100], F32, name=f"sr_{j}") for j in range(2)]
            er = [cp.tile([128, 100], F32, name=f"er_{j}") for j in range(2)]
            cm = [cp.tile([128, 100], BF16, name=f"cm_{j}") for j in range(2)]
            vsb = cp.tile([128, KT, 512], BF16, name="vsb")
            ysb = [cp.tile([128, PIX], BF16, name=f"ysb_{m}") for m in range(4)]
            y1 = cp.tile([128, ROWS, 102], BF16, name="y1")
            y2 = cp.tile([128, ROWS, 102], BF16, name="y2")
            y3 = cp.tile([128, 25 * T], BF16, name="y3")
            conf_s = cp.tile([2, 25 * T], F32, name="conf_s")
            start_s = cp.tile([1, T], F32, name="start_s")
            end_s = cp.tile([1, T], F32, name="end_s")

            for t_ in (base1[0], base1[1], basef[0], basef[1]):
                nc.vector.memset(t_[:, 0:1], 0.0)
                nc.vector.memset(t_[:, 101:102], 0.0)
            for t_ in (y1, y2):
                nc.vector.memset(t_[:, :, 0:1], 0.0)
                nc.vector.memset(t_[:, :, 101:102], 0.0)

            # ---------------- 1D conv stack: critical path to cm ----------------
            with tc.tile_pool(name="ps1d", bufs=4, space="PSUM") as pp:
                for j in range(2):
                    ps = pp.tile([128, 100], F32, name=f"ps_b1_{j}", tag="ps1d")
                    for g2 in range(2):
                        g = 2 * j + g2
                        for dt in range(3):
                            nc.tensor.matmul(
                                ps[64 * g2:64 * g2 + 64, :],
                                b1w[:, g, dt, :],
                                xg[:, g, dt:dt + 100],
                                start=(dt == 0), stop=(dt == 2))
                    nc.scalar.activation(base1[j][:, 1:101], ps[:], AF.Relu,
                                         bias=b1b[:, j:j + 1])
                for j in range(2):
                    ps = pp.tile([128, 100], F32, name=f"ps_b2_{j}", tag="ps1d")
                    for g2 in range(2):
                        sl = slice(64 * g2, 64 * g2 + 64)
                        for dt in range(3):
                            nc.tensor.matmul(
                                ps[sl, :],
                                b2w[sl, j, dt, :],
                                base1[j][sl, dt:dt + 100],
                                start=(dt == 0), stop=(dt == 2))
                    nc.scalar.activation(basef[j][:, 1:101], ps[:], AF.Relu,
                                         bias=b2b[:, j:j + 1])
                # p1 -> cm (bf16)
                for m in range(2):
                    ps = pp.tile([128, 100], F32, name=f"ps_p1_{m}", tag="ps1d")
                    for j in range(2):
                        for dt in range(3):
                            nc.tensor.matmul(
                                ps[:],
                                p1w[:, j, dt, 128 * m:128 * m + 128],
                                basef[j][:, dt:dt + 100],
                                start=(j == 0 and dt == 0), stop=(j == 1 and dt == 2))
                    nc.scalar.activation(cm[m][:], ps[:], AF.Relu,
                                         bias=p1b[:, m:m + 1])

                # ---- start/end branches: interleaved to overlap ACT chains ----
                ps_se = {}
                for bi, (wte, bte, dest) in enumerate(
                        ((s1w, s1b, sr), (e1w, e1b, er))):
                    for j in range(2):
                        ps = pp.tile([128, 100], F32, name=f"ps_se_{bi}_{j}",
                                     tag="ps1d")
                        ps_se[(bi, j)] = ps
                        for g2 in range(2):
                            sl = slice(64 * g2, 64 * g2 + 64)
                            for dt in range(3):
                                nc.tensor.matmul(
                                    ps[sl, :],
                                    wte[sl, j, dt, :],
                                    basef[j][sl, dt:dt + 100],
                                    start=(dt == 0), stop=(dt == 2))
                for bi, (bte, dest) in enumerate(((s1b, sr), (e1b, er))):
                    for j in range(2):
                        nc.scalar.activation(dest[j][:], ps_se[(bi, j)][:], AF.Relu,
                                             bias=bte[:, j:j + 1])
                ps1s = []
                for bi, (dest, w2e) in enumerate(((sr, s2w), (er, e2w))):
                    ps1 = pp.tile([1, 100], F32, name=f"ps_1_{bi}", tag="ps1", bufs=2)
                    ps1s.append(ps1)
                    for j in range(2):
                        nc.tensor.matmul(ps1[0:1, :], w2e[:, j:j + 1], dest[j][:],
                                         start=(j == 0), stop=(j == 1))
                for bi, (b2e, outdram, stile) in enumerate(
                        ((s2b, start_d, start_s), (e2b, end_d, end_s))):
                    nc.scalar.activation(stile[:], ps1s[bi][0:1, :], AF.Sigmoid,
                                         bias=b2e[0:1, 0:1])
                    nc.scalar.dma_start(out=outdram[:], in_=stile[:])

            # ------- big matmul: y = V^T @ W2, relu(+bias); V built in block 0 -------
            KC = 5    # k-tiles per W2 DMA
            NCHUNK = 4
            LA = 2    # V lookahead (k-tiles) in block 0
            with tc.tile_pool(name="w2st", bufs=4) as w2p, \
                 tc.tile_pool(name="psy", bufs=6, space="PSUM") as pyp, \
                 tc.tile_pool(name="p3dst", bufs=8) as p3p, \
                 tc.tile_pool(name="vstg", bufs=4) as vsp, \
                 tc.tile_pool(name="psv", bufs=2, space="PSUM") as pvp:
                p3ts = []
                for n0 in range(0, NS, NCHUNK):
                    p3t_ = p3p.tile([128, NCHUNK, 2, 512], BF16,
                                    name=f"p3t{n0}", tag="p3t")
                    nc.sync.dma_start(out=p3t_[:], in_=p3d_d[:, n0:n0 + NCHUNK, :, :])
                    p3ts.append(p3t_)

                def emit_v(n):
                    p3t = p3ts[n // NCHUNK]
                    psv = pvp.tile([100, 512], F32, name="psv", tag="psv")
                    for j in range(2):
                        nc.tensor.matmul(psv[0:100, :], cm[j][:, 0:100],
                                         p3t[:, n % NCHUNK, j, :],
                                         start=(j == 0), stop=(j == 1))
                    vst = vsp.tile([100, 512], BF16, name="vst", tag="vst")
                    nc.vector.tensor_copy(vst[:], psv[0:100, :])
                    nt0 = 100 * n
                    k0, p0 = divmod(nt0, 128)
                    seg = min(128 - p0, 100)
                    eng = (nc.scalar, nc.sync)[n % 2]
                    eng.dma_start(out=vsb[p0:p0 + seg, k0, :], in_=vst[0:seg, :])
                    if seg < 100:
                        eng.dma_start(out=vsb[0:100 - seg, k0 + 1, :],
                                      in_=vst[seg:100, :])

                navail = 0
                for blk in range(NBLK):
                    c0 = 512 * blk
                    N = min(512, PIX - c0)
                    psy = [pyp.tile([128, 512], F32, name=f"psy{m}", tag="psy")
                           for m in range(4)]
                    w2t = None
                    for k in range(KT):
                        while navail < NS and navail <= ((k + LA) * 128 + 127) // 100:
                            emit_v(navail)
                            navail += 1
                        if k % KC == 0:
                            w2t = w2p.tile([128, KC, 512], BF16, name="w2t", tag="w2t")
                            w2eng = nc.scalar if blk == 0 else nc.sync
                            w2eng.dma_start(
                                out=w2t[:],
                                in_=w2_d[blk, :, k:k + KC, :])
                        for m in range(4):
                            nc.tensor.matmul(
                                psy[m][:, 0:N],
                                vsb[:, k, 128 * m:128 * m + 128],
                                w2t[:, k % KC, 0:N],
                                start=(k == 0), stop=(k == KT - 1))
                    for m in range(4):
                        nc.scalar.activation(ysb[m][:, c0:c0 + N], psy[m][:, 0:N],
                                             AF.Relu, bias=p3db[:, m:m + 1])
                # zero out-of-image boundary rows (rows 0,1 and 27,28)
                for m in range(4):
                    nc.vector.tensor_scalar_mul(ysb[m][:, 0:200], ysb[m][:, 0:200],
                                                rmro[:, 0:1])
                    nc.vector.tensor_scalar_mul(ysb[m][:, 2700:2900],
                                                ysb[m][:, 2700:2900], rmro[:, 1:2])

            # ---------------- q1..q4 ----------------
            with tc.tile_pool(name="psq", bufs=6, space="PSUM") as pqp:
                # q1: 1x1, 512 -> 128, all 29 rows
                for c in range(6):
                    r0 = 5 * c
                    nr = min(5, ROWS - r0)
                    N = nr * 100
                    ps = pqp.tile([128, 500], F32, name="psq1", tag="psq")
                    for j in range(4):
                        nc.tensor.matmul(ps[:, 0:N], q1w[:, j, :],
                                         ysb[j][:, 100 * r0:100 * r0 + N],
                                         start=(j == 0), stop=(j == 3))
                    nc.scalar.activation(
                        y1[:, r0:r0 + nr, 1:101],
                        ps[:, 0:N].rearrange("p (r e) -> p r e", r=nr),
                        AF.Relu, bias=q1b[:, 0:1])
                nc.vector.tensor_scalar_mul(y1[:, 0:2, 1:101], y1[:, 0:2, 1:101],
                                            rmro[:, 0:1])
                nc.vector.tensor_scalar_mul(y1[:, 27:29, 1:101], y1[:, 27:29, 1:101],
                                            rmro[:, 1:2])
                # q2: 3x3 on y1 rows [0,29) -> y2 rows [1,28)
                chunks2 = [(1 + 5 * c, min(5, 28 - (1 + 5 * c))) for c in range(6)]
                ps2 = [pqp.tile([128, 500], F32, name=f"psq2_{c}", tag="psq")
                       for c in range(6)]
                for sh in range(9):
                    dr, dc = divmod(sh, 3)
                    for c, (r0, nr) in enumerate(chunks2):
                        N = nr * 100
                        nc.tensor.matmul(
                            ps2[c][:, 0:N],
                            q2w[:, sh, :],
                            y1[:, r0 + dr - 1:r0 + dr - 1 + nr, dc:dc + 100],
                            start=(sh == 0), stop=(sh == 8))
                for c, (r0, nr) in enumerate(chunks2):
                    N = nr * 100
                    nc.scalar.activation(
                        y2[:, r0:r0 + nr, 1:101],
                        ps2[c][:, 0:N].rearrange("p (r e) -> p r e", r=nr),
                        AF.Relu, bias=q2b[:, 0:1])
                nc.vector.tensor_scalar_mul(y2[:, 1:2, 1:101], y2[:, 1:2, 1:101],
                                            rmro[:, 0:1])
                nc.vector.tensor_scalar_mul(y2[:, 27:28, 1:101], y2[:, 27:28, 1:101],
                                            rmro[:, 1:2])
                # q3: 3x3 on y2 rows [1,28) -> y3 rows [2,27) (all valid)
                ps3 = [pqp.tile([128, 500], F32, name=f"psq3_{c}", tag="psq")
                       for c in range(5)]
                for sh in range(9):
                    dr, dc = divmod(sh, 3)
                    for c in range(5):
                        r0 = 2 + 5 * c
                        nc.tensor.matmul(
                            ps3[c][:, 0:500],
                            q3w[:, sh, :],
                            y2[:, r0 + dr - 1:r0 + dr - 1 + 5, dc:dc + 100],
                            start=(sh == 0), stop=(sh == 8))
                for c in range(5):
                    nc.scalar.activation(y3[:, 500 * c:500 * c + 500],
                                         ps3[c][:, 0:500], AF.Relu,
                                         bias=q3b[:, 0:1])
                # q4: 1x1 -> 2 ch, sigmoid
                for c in range(5):
                    ps4 = pqp.tile([2, 500], F32, name="psq4", tag="psq4", bufs=2)
                    nc.tensor.matmul(ps4[0:2, :], q4w[:, 0:2],
                                     y3[:, 500 * c:500 * c + 500],
                                     start=True, stop=True)
                    nc.scalar.activation(conf_s[0:2, 500 * c:500 * c + 500],
                                         ps4[0:2, :], AF.Sigmoid,
                                         bias=q4b[0:2, 0:1])
                nc.scalar.dma_start(out=conf_d[:], in_=conf_s[:])

    nc.compile()
    return nc


def _marshal(inputs):
    f32 = np.float32
    bf16 = ml_dtypes.bfloat16
    x = np.asarray(inputs["x"], f32)
    mask = np.asarray(inputs["sample_mask"], f32)

    xgs = []
    for b in range(B):
        xb = np.zeros((100, 4, 102), f32)
        xb[:, :, 1:101] = x[b].reshape(4, 100, 100).transpose(1, 0, 2)
        xgs.append(xb)

    b1w = np.zeros((100, 4, 3, 64), f32)
    w = np.asarray(inputs["b1_w"], f32)  # (256, 100, 3)
    for gi in range(4):
        b1w[:, gi, :, :] = w[64 * gi:64 * gi + 64].transpose(1, 2, 0)

    def group64(w):  # (256, 64, 3) -> (128, 2*3*64)
        out = np.zeros((128, 2, 3, 64), f32)
        for gi in range(4):
            out[64 * (gi % 2):64 * (gi % 2) + 64, gi // 2, :, :] = \
                w[64 * gi:64 * gi + 64].transpose(1, 2, 0)
        return out.reshape(128, -1)

    def bias2(b):
        return np.ascontiguousarray(np.asarray(b, f32).reshape(2, 128).T)

    pack0 = np.concatenate([bias2(inputs["b1_b"]), bias2(inputs["b2_b"]),
                            bias2(inputs["p1_b"])], axis=1)  # (128, 6)

    packf = np.zeros((128, NF), f32)

    def put(nm, arr):
        o, wdt = _PF[nm]
        packf[:arr.shape[0], o:o + wdt] = arr.reshape(arr.shape[0], wdt)

    put("b2w", group64(np.asarray(inputs["b2_w"], f32)))
    put("s1w", group64(np.asarray(inputs["s1_w"], f32)))
    put("e1w", group64(np.asarray(inputs["e1_w"], f32)))
    p1 = np.asarray(inputs["p1_w"], f32)  # (256, 256, 3)
    put("p1w", np.ascontiguousarray(
        p1.reshape(256, 2, 128, 3).transpose(2, 1, 3, 0)).reshape(128, -1))
    put("b1b", bias2(inputs["b1_b"]))
    put("b2b", bias2(inputs["b2_b"]))
    put("s1b", bias2(inputs["s1_b"]))
    put("e1b", bias2(inputs["e1_b"]))
    put("p1b", bias2(inputs["p1_b"]))
    put("s2w", np.ascontiguousarray(
        np.asarray(inputs["s2_w"], f32)[0, :, 0].reshape(2, 128).T))
    put("e2w", np.ascontiguousarray(
        np.asarray(inputs["e2_w"], f32)[0, :, 0].reshape(2, 128).T))
    packf[0, _PF["s2b"][0]] = np.asarray(inputs["s2_b"], f32).item()
    packf[0, _PF["e2b"][0]] = np.asarray(inputs["e2_b"], f32).item()
    put("p3db", np.ascontiguousarray(
        np.asarray(inputs["p3d_b"], f32).reshape(4, 128).T))
    packf[:, _PF["q1b"][0]] = np.asarray(inputs["q1_b"], f32)
    packf[:, _PF["q2b"][0]] = np.asarray(inputs["q2_b"], f32)
    packf[:, _PF["q3b"][0]] = np.asarray(inputs["q3_b"], f32)
    packf[0:2, _PF["q4b"][0]] = np.asarray(inputs["q4_b"], f32)

    packh = np.zeros((128, NH), bf16)
    q1 = np.asarray(inputs["q1_w"], f32)[:, :, 0, 0]  # (128, 512)
    packh[:, 0:512] = np.ascontiguousarray(
        q1.T.reshape(4, 128, 128).transpose(1, 0, 2)).reshape(128, 512).astype(bf16)
    for nm in ("q2", "q3"):
        qw = np.asarray(inputs[f"{nm}_w"], f32)
        o, wdt = _PH[f"{nm}w"]
        packh[:, o:o + wdt] = np.ascontiguousarray(
            qw.transpose(2, 3, 1, 0).reshape(9, 128, 128).transpose(1, 0, 2)
        ).reshape(128, wdt).astype(bf16)

    packh[:, 2816:2818] = np.ascontiguousarray(
        np.asarray(inputs["q4_w"], f32)[:, :, 0, 0].T).astype(bf16)

    p3 = np.asarray(inputs["p3d_w"], f32)[:, :, :, 0, 0]  # (512, 256, 32)
    p3dT = np.ascontiguousarray(
        p3.transpose(2, 1, 0).reshape(NS, 2, 128, 512).transpose(2, 0, 1, 3)
    ).astype(bf16)  # [c, n, j, o]

    w2_full = np.ascontiguousarray(
        mask.reshape(T, NS, T, T).transpose(1, 0, 2, 3)).reshape(NS * T, T, T)
    w2_bands, packfs = [], []
    for k in range(4):
        s_lo = 25 * k - 2
        w2c = np.zeros((NS * T, ROWS, T), f32)
        lo, hi = max(s_lo, 0), min(s_lo + ROWS, T)
        w2c[:, lo - s_lo:hi - s_lo, :] = w2_full[:, lo:hi, :]
        w2kt = w2c.reshape(KT, 128, PIX)
        w2b = np.zeros((NBLK, 128, KT, 512), np.float32)
        for blk in range(NBLK):
            c0 = 512 * blk
            n_ = min(512, PIX - c0)
            w2b[blk, :, :, 0:n_] = w2kt[:, :, c0:c0 + n_].transpose(1, 0, 2)
        w2_bands.append(w2b.astype(bf16))
        pfk = packf.copy()
        pfk[:, _PF["rmrows"][0]] = 1.0 if k > 0 else 0.0
        pfk[:, _PF["rmrows"][0] + 1] = 1.0 if k < 3 else 0.0
        packfs.append(pfk)

    in_maps = []
    for c in range(8):
        b, k = divmod(c, 4)
        in_maps.append({
            "pack0": pack0, "xg": xgs[b], "b1w": b1w, "packf": packfs[k],
            "packh": packh, "p3d": p3dT, "w2": w2_bands[k],
        })
    return in_maps


def kernel(**inputs):
    global _COMPILED
    from concourse.bass_utils import run_bass_kernel_spmd

    if _COMPILED is None:
        _COMPILED = _build()
    nc = _COMPILED

    in_maps = _marshal(inputs)
    res = run_bass_kernel_spmd(nc, in_maps, core_ids=list(range(8)),
                               trace=False)

    conf = np.zeros((B, 2, T, T), np.float32)
    start = np.zeros((B, T), np.float32)
    end = np.zeros((B, T), np.float32)
    for c in range(8):
        b, k = divmod(c, 4)
        r = res.results[c]
        conf[b, :, 25 * k:25 * k + 25, :] = r["conf_o"].reshape(2, 25, T)
        if k == 0:
            start[b] = r["start_o"][0]
            end[b] = r["end_o"][0]
    return conf, start, end
